# revision 13
# baseline (speedup 1.0000x reference)
"""Trainium2 Bass kernel for CrossStockAttention (sparse similarity-top-k attention).

Sharding: 8 cores = 2 batches x 4 query-row blocks of 512. Each core receives
the full key set of its batch plus its query slice, computes sim -> chunked
top-40 threshold -> mask -> QKV -> masked softmax attention -> out-proj ->
residual -> LayerNorm for its 512 rows on-chip.

v4 (from the 166us v3):
 - PE filler matmuls (1 x N=512 per key-tile) bridge the exp-paced PE idle
   gaps so the HAM un-throttles; v3 ran the whole stream at K=4/8 (600ns
   matmuls) because PE bursts never reached a full 3.4us activity window
 - zero-bias / identity-LN specialization decided at kernel() time from the
   actual inputs (spec fills are zeros/ones) -> no bias ts ops, no LN g/b
   tensor_tensor passes; non-zero inputs rebuild with the general path
 - dummy sqrt removed (Tile hoisted it early, evicting the exp ACT table
   mid-front and forcing a reload before the first real exp)
 - sim emitted jg-major with K/Q ec0 projections between, weights DMA moved
   to the gpsimd queue, nT split sync/scalar -> first exp ~14us earlier
 - ec0 po evacuation interleaved into the early ec1 mask stream (hp0 first)
   instead of one 5us DVE block at the boundary
"""

import numpy as np

B, N, D, H = 2, 2048, 256, 8
DH = D // H            # 32
TOPK = 40
P = 128
NCORES = 8
QS = 512               # query rows per core
NT = N // P            # 16 key row-tiles
QT = QS // P           # 4 query row-tiles
DC = D // P            # 2 contraction chunks of 128
LN_EPS = 1e-5
SCALE = 1.0 / DH ** 0.5
MR_MIN = -60000.0      # fp16-safe
TPAD = -100.0

CH = 256               # topk chunk width (8 chunks, 64 candidates)
NCH = N // CH
NSLOT = 32             # exp slab slots: full double-slab per ec group
GPS_JTS = (1, 3, 5, 7, 9, 11, 13)   # key-tiles whose hp0 mask-mult runs on gpsimd
N_WARM = 32
AV_LAG = 10

_CACHE = {}


def _emit(nc, tc, ctx, zb_qk, zb_o, ln_id):
    import concourse.bass as bass
    import concourse.mybir as mybir
    from concourse.masks import make_identity

    f32 = mybir.dt.float32
    f16 = mybir.dt.float16
    AF = mybir.ActivationFunctionType
    OP = mybir.AluOpType

    xq_d = nc.dram_tensor("xq", [QS, D], f32, kind="ExternalInput")
    xt16_d = nc.dram_tensor("xt16", [D, N], f16, kind="ExternalInput")
    xqt16_d = nc.dram_tensor("xqt16", [D, QS], f16, kind="ExternalInput")
    ntT_d = nc.dram_tensor("ntT", [D, N], f16, kind="ExternalInput")
    nqtT_d = nc.dram_tensor("nqtT", [D, QS], f16, kind="ExternalInput")
    w_d = {}
    for nm in ("wq", "wk", "wv", "wo"):
        w_d[nm] = nc.dram_tensor(nm + "16", [D, D], f16, kind="ExternalInput")
    bkP_d = nc.dram_tensor("bkP", [P, DC], f32, kind="ExternalInput")
    bqP_d = nc.dram_tensor("bqP", [P, DC], f32, kind="ExternalInput")
    boP_d = nc.dram_tensor("boP", [P, DC], f32, kind="ExternalInput")
    qv_d = nc.dram_tensor("qv", [P, QT], f32, kind="ExternalInput")
    g_d = nc.dram_tensor("g", [D], f32, kind="ExternalInput")
    bt_d = nc.dram_tensor("bt", [D], f32, kind="ExternalInput")
    out_d = nc.dram_tensor("out", [QS, D], f32, kind="ExternalOutput")

    def bcast_ap(handle, n_part):
        ap = handle.ap()
        return bass.AP(tensor=ap.tensor, offset=ap.offset,
                       ap=[[0, n_part]] + [list(p) for p in ap.ap])

    def rep_ap(ap_in, ins_pos, count):
        a = [list(p) for p in ap_in.ap]
        a.insert(ins_pos, [0, count])
        return bass.AP(tensor=ap_in.tensor, offset=ap_in.offset, ap=a)

    consts = ctx.enter_context(tc.tile_pool(name="consts", bufs=1))
    big = ctx.enter_context(tc.tile_pool(name="big", bufs=1))
    scrp = ctx.enter_context(tc.tile_pool(name="scrp", bufs=2))
    small = ctx.enter_context(tc.tile_pool(name="small", bufs=6))
    psA = ctx.enter_context(tc.tile_pool(name="psA", bufs=2, space="PSUM"))
    psV = ctx.enter_context(tc.tile_pool(name="psV", bufs=2, space="PSUM"))
    psT = ctx.enter_context(tc.tile_pool(name="psT", bufs=2, space="PSUM"))

    # ---------------- constants / warmup / input DMAs ----------------
    ident = consts.tile([P, P], f32, tag="ident")
    make_identity(nc, ident)
    ident16 = consts.tile([P, P], f16, tag="ident16")
    nc.vector.tensor_copy(ident16, ident)

    for w in range(N_WARM):
        wt = psT.tile([P, 4, P], f32, tag="psT", name=f"warm_{w}", bufs=2)
        nc.tensor.matmul(wt[:, 0, :], lhsT=ident16, rhs=ident16,
                         start=True, stop=True)

    nfill = [0]

    def filler(ec):
        # real matmuls (no deps beyond resident kT) to keep the HAM warm
        ft = psT.tile([P, 4, P], f32, tag="psT", name=f"fill_{nfill[0]}", bufs=2)
        nfill[0] += 1
        nc.tensor.matmul(ft.rearrange("p a b -> p (a b)")[:, 0:256], lhsT=ident16,
                         rhs=kT[:, ec, 0:256], start=True, stop=True)
        nc.tensor.matmul(ft.rearrange("p a b -> p (a b)")[:, 256:512], lhsT=ident16,
                         rhs=kT[:, ec, 256:512], start=True, stop=True)

    # sim operands: nqT + first key half on sync, second half on scalar
    nqT = big.tile([P, DC, QS], f16, tag="nqT")
    nc.sync.dma_start(out=nqT, in_=nqtT_d.ap().rearrange("(dc p) j -> p dc j", p=P))
    nT = big.tile([P, DC, N], f16, tag="nT")
    n_r = ntT_d.ap().rearrange("(dc p) j -> p dc j", p=P)
    nc.sync.dma_start(out=nT[:, :, 0:1024], in_=n_r[:, :, 0:1024])
    nc.scalar.dma_start(out=nT[:, :, 1024:2048], in_=n_r[:, :, 1024:2048])
    xT = big.tile([P, DC, N], f16, tag="xT")
    nc.scalar.dma_start(out=xT, in_=xt16_d.ap().rearrange("(dc p) j -> p dc j", p=P))
    xqT = big.tile([P, DC, QS], f16, tag="xqT")
    nc.scalar.dma_start(out=xqT, in_=xqt16_d.ap().rearrange("(dc p) j -> p dc j", p=P))
    w16 = {}
    for nm in ("wk", "wq", "wv", "wo"):
        w16[nm] = consts.tile([P, DC, D], f16, tag=f"w16_{nm}", name=f"w16_{nm}")
        nc.gpsimd.dma_start(out=w16[nm],
                            in_=w_d[nm].ap().rearrange("(dc p) d -> p dc d", p=P))
    bkP = consts.tile([P, DC], f32, tag="bkP")
    nc.gpsimd.dma_start(out=bkP, in_=bkP_d[:, :])
    bqP = consts.tile([P, DC], f32, tag="bqP")
    nc.gpsimd.dma_start(out=bqP, in_=bqP_d[:, :])
    boP = consts.tile([P, DC], f32, tag="boP")
    nc.gpsimd.dma_start(out=boP, in_=boP_d[:, :])
    qv_sb = consts.tile([P, QT], f32, tag="qv_sb")
    nc.gpsimd.dma_start(out=qv_sb, in_=qv_d[:, :])
    g_rep = consts.tile([P, D], f32, tag="g_rep")
    nc.gpsimd.dma_start(out=g_rep, in_=bcast_ap(g_d, P))
    bt_rep = consts.tile([P, D], f32, tag="bt_rep")
    nc.gpsimd.dma_start(out=bt_rep, in_=bcast_ap(bt_d, P))
    xq_rows = big.tile([P, QT, D], f32, tag="xq_rows")
    nc.gpsimd.dma_start(out=xq_rows, in_=xq_d.ap().rearrange("(t p) d -> p t d", p=P))

    v_aug = big.tile([P, NT, H, DH + 1], f16, tag="v_aug")
    nc.gpsimd.memset(v_aug[:, :, :, DH:DH + 1], 1.0)

    dumm = small.tile([P, 8], f32, tag="dumm")
    nc.scalar.activation(dumm, ident[:, 0:8], AF.Exp)   # exp table preload

    # ---------------- sim (jg-major front half, K/Q ec0 between) -----
    kT = big.tile([P, DC, N], f16, tag="kT")
    qT = big.tile([P, DC, QS], f16, tag="qT")
    sims = [big.tile([P, N], f16, tag=f"sim_{t}", name=f"sim_{t}")
            for t in range(QT)]
    sim_ps = {}

    def sim_mm(t):
        # 4 single-bank psV tiles per query tile; copies run on DVE
        for jc in range(4):
            ps = psV.tile([P, 512], f32, tag="psV", name=f"psim_{t}_{jc}")
            sim_ps[(t, jc)] = ps
            for dc in range(DC):
                nc.tensor.matmul(
                    ps,
                    lhsT=nqT[:, dc, t * P:(t + 1) * P],
                    rhs=nT[:, dc, jc * 512:(jc + 1) * 512],
                    start=dc == 0, stop=dc == DC - 1)

    def sim_cp(t):
        for jc in range(4):
            nc.vector.tensor_copy(sims[t][:, jc * 512:(jc + 1) * 512],
                                  sim_ps[(t, jc)])

    def kproj_mm(ec, jg):
        pk = psA.tile([P, 2, 512], f32, tag="psA", name=f"pk_{ec}_{jg}")
        for k in range(2):
            jc = jg * 2 + k
            for dc in range(DC):
                nc.tensor.matmul(
                    pk[:, k, :],
                    lhsT=w16["wk"][:, dc, ec * P:(ec + 1) * P],
                    rhs=xT[:, dc, jc * 512:(jc + 1) * 512],
                    start=dc == 0, stop=dc == DC - 1)
        return pk

    def qproj_mm(ec):
        pq = psA.tile([P, 2, 512], f32, tag="psA", name=f"pq_{ec}")
        for dc in range(DC):
            nc.tensor.matmul(
                pq[:, 0, :],
                lhsT=w16["wq"][:, dc, ec * P:(ec + 1) * P],
                rhs=xqT[:, dc, :],
                start=dc == 0, stop=dc == DC - 1)
        return pq

    def kq_bias(ec):
        if not zb_qk:
            nc.gpsimd.tensor_scalar(kT[:, ec, :], kT[:, ec, :], bkP[:, ec:ec + 1],
                                    None, op0=OP.add)
            nc.gpsimd.tensor_scalar(qT[:, ec, :], qT[:, ec, :], bqP[:, ec:ec + 1],
                                    None, op0=OP.add)

    sim_mm(0)
    sim_mm(1)
    pks0 = [kproj_mm(0, jg) for jg in range(2)]
    pq0 = qproj_mm(0)
    for jg in range(2):
        nc.scalar.copy(kT[:, 0, jg * 1024:(jg + 1) * 1024], pks0[jg])
    nc.scalar.copy(qT[:, 0, :], pq0[:, 0, :])
    kq_bias(0)

    # ---------------- V projection ----------------
    pvs = {}

    def v_mm(jt):
        pv = psV.tile([P, 512], f32, tag="psV", name=f"pv_{jt}")
        pvs[jt] = pv
        for dc in range(DC):
            nc.tensor.matmul(
                pv[:, 0:D],
                lhsT=xT[:, dc, jt * P:(jt + 1) * P],
                rhs=w16["wv"][:, dc, :],
                start=dc == 0, stop=dc == DC - 1)

    def v_evac(jt):
        nc.vector.tensor_copy(v_aug[:, jt, :, 0:DH],
                              pvs[jt][:, 0:D].rearrange("p (h d) -> p h d", h=H))

    # ---------------- topk threshold + mask ----------------
    maskT = big.tile([P, NT, QS], f16, tag="maskT")

    def topk_tile(t):
        sim_m = sims[t]
        cand = scrp.tile([P, 64], f16, tag="cand", name=f"cand_{t}")
        for c in range(NCH):
            nc.vector.max(cand[:, c * 8:c * 8 + 8], sim_m[:, c * CH:(c + 1) * CH])
        mscr = scrp.tile([P, 64], f16, tag="mscr", name=f"mscr_{t}")
        mx = None
        for it in range(5):
            mx = small.tile([P, 8], f16, tag="mx8", name=f"mx_{t}_{it}")
            src = cand if it == 0 else mscr
            nc.vector.max(mx, src)
            if it < 4:
                nc.vector.match_replace(mscr, mx, src, MR_MIN)
        tS = small.tile([P, 1], f32, tag="tS", name=f"tS_{t}")
        tP_ = small.tile([P, 1], f32, tag="tP", name=f"tP_{t}")
        nc.vector.tensor_scalar(tP_, qv_sb[:, t:t + 1], 1.0, float(-TPAD),
                                op0=OP.subtract, op1=OP.mult)
        nc.vector.tensor_mul(tS, mx[:, 7:8], qv_sb[:, t:t + 1])
        nc.vector.tensor_add(tS, tS, tP_)
        mrow = scrp.tile([P, N], f16, tag="mrow", name=f"mrow_{t}")
        nc.vector.tensor_scalar(mrow, sim_m, tS, None, op0=OP.is_ge)
        nc.sync.dma_start_transpose(maskT[:, :, t * P:(t + 1) * P], mrow)

    # ---------------- scores/exp/mask/AV pipeline --------------------
    et = big.tile([P, NSLOT, 2, 512], f16, tag="et")
    outT = big.tile([P, DC, QS], f16, tag="outT")
    sumsAB = [big.tile([P, QS], f32, tag=f"sums_{e}", name=f"sums_{e}")
              for e in range(2)]
    pos = {}

    def slot(ec, jt, hp):
        return (ec * 2 * NT + 2 * jt + hp) % NSLOT

    def sc_exp(ec, jt, hp):
        ps = psA.tile([P, 2, 512], f32, tag="psA", name=f"sc_{ec}_{jt}_{hp}")
        for hl in range(2):
            r0 = 64 * hp + 32 * hl
            nc.tensor.matmul(
                ps[:, hl, :],
                lhsT=kT[r0:r0 + 32, ec, jt * P:(jt + 1) * P],
                rhs=qT[r0:r0 + 32, ec, :],
                start=True, stop=True, tile_position=(r0, 0))
        s = slot(ec, jt, hp)
        nc.scalar.activation(et[:, s, :, :], ps, AF.Exp, scale=float(SCALE))

    def mask_mult(ec, jt):
        m_b = rep_ap(maskT[:, jt, :], 1, 2)
        for hp in range(2):
            eng = nc.gpsimd if (hp == 0 and jt in GPS_JTS) else nc.vector
            s = slot(ec, jt, hp)
            eng.tensor_mul(et[:, s, :, :], et[:, s, :, :], m_b)

    def av(ec, jt):
        for hp in range(2):
            po = pos[(ec, hp)]
            s = slot(ec, jt, hp)
            for hl in range(2):
                h = 4 * ec + 2 * hp + hl
                nc.tensor.matmul(
                    po[64 * hl:64 * hl + DH + 1, :],
                    lhsT=v_aug[:, jt, h, :],
                    rhs=et[:, s, hl, :],
                    start=jt == 0, stop=jt == NT - 1,
                    tile_position=(0, 64 * hl), skip_group_check=True)

    def ec_evac_copies(ec, eng):
        # 8 copies: [hp0 outT+sums x2, then hp1], all shifts 32-aligned
        cps = []
        sums = sumsAB[ec]
        for hp in range(2):
            po = pos[(ec, hp)]
            for hl in range(2):
                hq = 2 * hp + hl
                cps.append((outT[32 * hq:32 * hq + 32, ec, :],
                            po[64 * hl:64 * hl + 32, :]))
                cps.append((sums[32 * hq:32 * hq + 1, :],
                            po[64 * hl + 32:64 * hl + 33, :]))
        return [(eng, dst, src) for dst, src in cps]

    sums_rows = big.tile([P, QT, H], f32, tag="sums_rows")
    recip_rows = big.tile([P, QT, H], f32, tag="recip_rows")
    out_rows = [big.tile([P, QT, P], f16, tag=f"our_{e}", name=f"our_{e}")
                for e in range(2)]
    outN = big.tile([P, DC, QS], f16, tag="outN")

    def ec_rows(ec):
        pt_s = psT.tile([P, 4, P], f32, tag="psT", name=f"pt_s_{ec}")
        for it in range(QT):
            nc.tensor.transpose(pt_s[:, it, :], sumsAB[ec][:, it * P:(it + 1) * P],
                                ident)
        for it in range(QT):
            base = pt_s[:, it, :]
            src = bass.AP(tensor=base.tensor, offset=base.offset,
                          ap=[list(base.ap[0]), [DH, 4]])
            nc.scalar.copy(sums_rows[:, it, 4 * ec:4 * ec + 4], src)
        pt_o = psT.tile([P, 4, P], f16, tag="psT", name=f"pt_o_{ec}")
        for it in range(QT):
            nc.tensor.transpose(pt_o[:, it, :],
                                outT[:, ec, it * P:(it + 1) * P], ident16)
        nc.vector.tensor_copy(out_rows[ec], pt_o)

    def ec_norm(ec):
        nc.vector.reciprocal(recip_rows[:, :, 4 * ec:4 * ec + 4],
                             sums_rows[:, :, 4 * ec:4 * ec + 4])
        rb = recip_rows[:, :, 4 * ec:4 * ec + 4]
        rb_b = bass.AP(tensor=rb.tensor, offset=rb.offset,
                       ap=[list(rb.ap[0]), list(rb.ap[1]), list(rb.ap[2]), [0, DH]])
        o = out_rows[ec]
        o_v = bass.AP(tensor=o.tensor, offset=o.offset,
                      ap=[list(o.ap[0]), list(o.ap[1]), [DH, 4], [1, DH]])
        nc.vector.tensor_mul(o_v, o_v, rb_b)

    def ec_outN(ec):
        pt = psT.tile([P, 4, P], f16, tag="psT", name=f"pt_n_{ec}")
        for it in range(QT):
            nc.tensor.transpose(pt[:, it, :], out_rows[ec][:, it, :], ident16)
        nc.scalar.copy(outN[:, ec, :], pt)

    # -- emission: ec0 stream --
    # DVE order: sim copies (t0,t1) first; sims t2/t3 MMs emitted at j0==2
    sim_cp(0)
    sim_cp(1)
    for j0 in range(AV_LAG):
        sc_exp(0, j0, 0); sc_exp(0, j0, 1)
        if j0 == 2:
            sim_mm(2)
            sim_mm(3)
            sim_cp(2)
            sim_cp(3)
            for t in range(QT):
                topk_tile(t)
        if 3 <= j0 <= 8:
            for jv in range(3 * (j0 - 3), min(3 * (j0 - 3) + 3, NT)):
                v_mm(jv)
        if j0 == 8:
            v_mm(15)
        if j0 == 4:
            pks1 = [kproj_mm(1, jg) for jg in range(2)]
            pq1 = qproj_mm(1)
            for jg in range(2):
                nc.scalar.copy(kT[:, 1, jg * 1024:(jg + 1) * 1024], pks1[jg])
            nc.scalar.copy(qT[:, 1, :], pq1[:, 0, :])
            kq_bias(1)
    for hp in range(2):
        pos[(0, hp)] = psV.tile([P, 512], f32, tag="psV", name=f"po_0_{hp}")
    for jt in range(NT):
        if jt + AV_LAG < NT:
            sc_exp(0, jt + AV_LAG, 0); sc_exp(0, jt + AV_LAG, 1)
        if jt < 4:
            for jv in range(4 * jt, 4 * jt + 4):
                v_evac(jv)
        mask_mult(0, jt)
        av(0, jt)
        filler(0)
    for hp in range(2):
        pos[(1, hp)] = psV.tile([P, 512], f32, tag="psV", name=f"po_1_{hp}")
    evacs0 = ec_evac_copies(0, "dve")
    for j0 in range(AV_LAG):
        sc_exp(1, j0, 0); sc_exp(1, j0, 1)
    for jt in range(NT):
        if jt + AV_LAG < NT:
            sc_exp(1, jt + AV_LAG, 0); sc_exp(1, jt + AV_LAG, 1)
        mask_mult(1, jt)
        # spread the ec0 po evacuation through the early ec1 mask stream
        if jt < 4:
            for _, dst, src in evacs0[2 * jt:2 * jt + 2]:
                nc.vector.tensor_copy(dst, src)
        av(1, jt)
        filler(1)
        if jt == 4:
            ec_rows(0)
        elif jt == 6:
            ec_norm(0)
        elif jt == 8:
            ec_outN(0)
    for eng, dst, src in ec_evac_copies(1, "act"):
        nc.scalar.copy(dst, src)
    ec_rows(1)
    ec_norm(1)
    ec_outN(1)

    # ---------------- out-proj, residual, LN --------------
    finalT = big.tile([P, DC, QS], f32, tag="finalT")
    pf = psA.tile([P, 2, 512], f32, tag="psA", name="pf")
    for eo in range(DC):
        for dc in range(DC):
            nc.tensor.matmul(
                pf[:, eo, :],
                lhsT=w16["wo"][:, dc, eo * P:(eo + 1) * P],
                rhs=outN[:, dc, :],
                start=dc == 0, stop=dc == DC - 1)
        nc.scalar.copy(finalT[:, eo, :], pf[:, eo, :])
        if not zb_o:
            nc.vector.tensor_scalar(finalT[:, eo, :], finalT[:, eo, :],
                                    boP[:, eo:eo + 1], None, op0=OP.add)

    fin = big.tile([P, QT, D], f32, tag="fin")
    for eo in range(DC):
        pt = psT.tile([P, 4, P], f32, tag="psT", name=f"ptf_{eo}")
        for it in range(QT):
            nc.tensor.transpose(pt[:, it, :], finalT[:, eo, it * P:(it + 1) * P],
                                ident)
        nc.scalar.copy(fin[:, 0:QT, eo * P:(eo + 1) * P], pt)
    nc.vector.tensor_add(fin, fin, xq_rows)

    st6 = small.tile([P, QT, 6], f32, tag="st6")
    mv = small.tile([P, QT, 2], f32, tag="mv")
    for t in range(QT):
        nc.vector.bn_stats(st6[:, t, :], fin[:, t, :])
        nc.vector.bn_aggr(mv[:, t, :], st6[:, t, :])
    rstd = small.tile([P, QT, 1], f32, tag="rstd")
    nc.vector.tensor_scalar(rstd, mv[:, :, 1:2], float(LN_EPS), None, op0=OP.add)
    nc.scalar.activation(rstd, rstd, AF.Sqrt)
    nc.vector.reciprocal(rstd, rstd)
    for t in range(QT):
        nc.vector.tensor_scalar(fin[:, t, :], fin[:, t, :], mv[:, t, 0:1],
                                rstd[:, t, 0:1], op0=OP.subtract, op1=OP.mult)
        if not ln_id:
            nc.vector.tensor_mul(fin[:, t, :], fin[:, t, :], g_rep)
            nc.vector.tensor_add(fin[:, t, :], fin[:, t, :], bt_rep)
        nc.sync.dma_start(out=out_d[t * P:(t + 1) * P, :], in_=fin[:, t, :])


def build_nc(zb_qk=True, zb_o=True, ln_id=True):
    from contextlib import ExitStack
    import concourse.bacc as bacc
    from concourse.tile import TileContext

    nc = bacc.Bacc("TRN2", target_bir_lowering=False, debug=False, num_devices=NCORES)
    with TileContext(nc) as tc:
        with ExitStack() as ctx:
            _emit(nc, tc, ctx, zb_qk, zb_o, ln_id)
    nc.compile()
    return nc


def _in_maps(inputs):
    x = np.ascontiguousarray(np.asarray(inputs["stock_features"], dtype=np.float32))
    valid = np.asarray(inputs["stock_valid_mask"]).astype(bool)
    wq = np.asarray(inputs["w_q"], np.float32)
    wk = np.asarray(inputs["w_k"], np.float32)
    wv = np.asarray(inputs["w_v"], np.float32)
    wo = np.asarray(inputs["w_o"], np.float32)
    bo_f = (np.asarray(inputs["b_v"], np.float32) @ wo
            + np.asarray(inputs["b_o"], np.float32))
    shared = {
        "wq16": np.ascontiguousarray(wq.astype(np.float16)),
        "wk16": np.ascontiguousarray(wk.astype(np.float16)),
        "wv16": np.ascontiguousarray(wv.astype(np.float16)),
        "wo16": np.ascontiguousarray(wo.astype(np.float16)),
        "bkP": np.ascontiguousarray(
            np.asarray(inputs["b_k"], np.float32).reshape(DC, P).T),
        "bqP": np.ascontiguousarray(
            np.asarray(inputs["b_q"], np.float32).reshape(DC, P).T),
        "boP": np.ascontiguousarray(bo_f.reshape(DC, P).T),
        "g": np.ascontiguousarray(inputs["ln_g"], np.float32),
        "bt": np.ascontiguousarray(inputs["ln_b"], np.float32),
    }
    maps = []
    for c in range(NCORES):
        b, qi = divmod(c, 4)
        q0 = qi * QS
        qv = valid[b, q0:q0 + QS].astype(np.float32).reshape(QT, P).T.copy()
        m = dict(shared)
        m["xq"] = np.ascontiguousarray(x[b, q0:q0 + QS])
        xt16 = np.ascontiguousarray(x[b].T.astype(np.float16))
        m["xt16"] = xt16
        m["xqt16"] = np.ascontiguousarray(xt16[:, q0:q0 + QS])
        n = x[b] / np.clip(np.linalg.norm(x[b], axis=-1, keepdims=True), 1e-12, None)
        n[~valid[b]] = 0.0
        ntT = np.ascontiguousarray(n.T.astype(np.float16))
        m["ntT"] = ntT
        m["nqtT"] = np.ascontiguousarray(ntT[:, q0:q0 + QS])
        m["qv"] = qv
        maps.append(m)
    return maps


def kernel(**inputs):
    from concourse.bass_utils import run_bass_kernel_spmd

    zb_qk = (not np.any(np.asarray(inputs["b_q"]))
             and not np.any(np.asarray(inputs["b_k"])))
    bo_f = (np.asarray(inputs["b_v"], np.float32)
            @ np.asarray(inputs["w_o"], np.float32)
            + np.asarray(inputs["b_o"], np.float32))
    zb_o = not np.any(bo_f)
    ln_id = (np.all(np.asarray(inputs["ln_g"]) == 1.0)
             and not np.any(np.asarray(inputs["ln_b"])))
    key = ("nc", zb_qk, zb_o, ln_id)
    if key not in _CACHE:
        _CACHE[key] = build_nc(zb_qk, zb_o, ln_id)
    nc = _CACHE[key]
    res = run_bass_kernel_spmd(nc, _in_maps(inputs), list(range(NCORES)))
    out = np.empty((B, N, D), np.float32)
    for c in range(NCORES):
        b, qi = divmod(c, 4)
        out[b, qi * QS:(qi + 1) * QS] = res.results[c]["out"]
    return out


# revision 15
# speedup vs baseline: 1.0379x; 1.0379x over previous
"""Trainium2 Bass kernel for CrossStockAttention (sparse similarity-top-k attention).

Sharding: 8 cores = 2 batches x 4 query-row blocks of 512. Each core receives
the full key set of its batch plus its query slice, computes sim -> chunked
top-40 threshold -> mask -> QKV -> masked softmax attention -> out-proj ->
residual -> LayerNorm for its 512 rows on-chip.

v4 (from the 166us v3):
 - PE filler matmuls (1 x N=512 per key-tile) bridge the exp-paced PE idle
   gaps so the HAM un-throttles; v3 ran the whole stream at K=4/8 (600ns
   matmuls) because PE bursts never reached a full 3.4us activity window
 - zero-bias / identity-LN specialization decided at kernel() time from the
   actual inputs (spec fills are zeros/ones) -> no bias ts ops, no LN g/b
   tensor_tensor passes; non-zero inputs rebuild with the general path
 - dummy sqrt removed (Tile hoisted it early, evicting the exp ACT table
   mid-front and forcing a reload before the first real exp)
 - sim emitted jg-major with K/Q ec0 projections between, weights DMA moved
   to the gpsimd queue, nT split sync/scalar -> first exp ~14us earlier
 - ec0 po evacuation interleaved into the early ec1 mask stream (hp0 first)
   instead of one 5us DVE block at the boundary
"""

import numpy as np

B, N, D, H = 2, 2048, 256, 8
DH = D // H            # 32
TOPK = 40
P = 128
NCORES = 8
QS = 512               # query rows per core
NT = N // P            # 16 key row-tiles
QT = QS // P           # 4 query row-tiles
DC = D // P            # 2 contraction chunks of 128
LN_EPS = 1e-5
SCALE = 1.0 / DH ** 0.5
MR_MIN = -60000.0      # fp16-safe
TPAD = -100.0

CH = 256               # topk chunk width (8 chunks, 64 candidates)
NCH = N // CH
NSLOT = 40             # exp slab slots (20 key-tiles in flight)
GPS_JTS = {0: (3, 9), 1: (1, 3, 5, 7, 9, 11, 13)}  # hp0 mask-mult on gpsimd
N_WARM = 32
AV_LAG = 10

_CACHE = {}


def _emit(nc, tc, ctx, zb_qk, zb_o, ln_id):
    import concourse.bass as bass
    import concourse.mybir as mybir
    from concourse.masks import make_identity

    f32 = mybir.dt.float32
    f16 = mybir.dt.float16
    AF = mybir.ActivationFunctionType
    OP = mybir.AluOpType

    xq_d = nc.dram_tensor("xq", [QS, D], f32, kind="ExternalInput")
    xt16_d = nc.dram_tensor("xt16", [D, N], f16, kind="ExternalInput")
    xqt16_d = nc.dram_tensor("xqt16", [D, QS], f16, kind="ExternalInput")
    ntT_d = nc.dram_tensor("ntT", [D, N], f16, kind="ExternalInput")
    nqtT_d = nc.dram_tensor("nqtT", [D, QS], f16, kind="ExternalInput")
    w_d = {}
    for nm in ("wq", "wk", "wv", "wo"):
        w_d[nm] = nc.dram_tensor(nm + "16", [D, D], f16, kind="ExternalInput")
    bkP_d = nc.dram_tensor("bkP", [P, DC], f32, kind="ExternalInput")
    bqP_d = nc.dram_tensor("bqP", [P, DC], f32, kind="ExternalInput")
    boP_d = nc.dram_tensor("boP", [P, DC], f32, kind="ExternalInput")
    qv_d = nc.dram_tensor("qv", [P, QT], f32, kind="ExternalInput")
    g_d = nc.dram_tensor("g", [D], f32, kind="ExternalInput")
    bt_d = nc.dram_tensor("bt", [D], f32, kind="ExternalInput")
    out_d = nc.dram_tensor("out", [QS, D], f32, kind="ExternalOutput")

    def bcast_ap(handle, n_part):
        ap = handle.ap()
        return bass.AP(tensor=ap.tensor, offset=ap.offset,
                       ap=[[0, n_part]] + [list(p) for p in ap.ap])

    def rep_ap(ap_in, ins_pos, count):
        a = [list(p) for p in ap_in.ap]
        a.insert(ins_pos, [0, count])
        return bass.AP(tensor=ap_in.tensor, offset=ap_in.offset, ap=a)

    consts = ctx.enter_context(tc.tile_pool(name="consts", bufs=1))
    big = ctx.enter_context(tc.tile_pool(name="big", bufs=1))
    scrp = ctx.enter_context(tc.tile_pool(name="scrp", bufs=2))
    small = ctx.enter_context(tc.tile_pool(name="small", bufs=6))
    psA = ctx.enter_context(tc.tile_pool(name="psA", bufs=2, space="PSUM"))
    psV = ctx.enter_context(tc.tile_pool(name="psV", bufs=2, space="PSUM"))
    psT = ctx.enter_context(tc.tile_pool(name="psT", bufs=2, space="PSUM"))

    # ---------------- constants / warmup / input DMAs ----------------
    ident = consts.tile([P, P], f32, tag="ident")
    make_identity(nc, ident)
    ident16 = consts.tile([P, P], f16, tag="ident16")
    nc.vector.tensor_copy(ident16, ident)

    for w in range(N_WARM):
        wt = psT.tile([P, 4, P], f32, tag="psT", name=f"warm_{w}", bufs=2)
        nc.tensor.matmul(wt[:, 0, :], lhsT=ident16, rhs=ident16,
                         start=True, stop=True)

    nfill = [0]

    def filler(ec):
        # real matmuls (no deps beyond resident kT) to keep the HAM warm
        ft = psT.tile([P, 4, P], f32, tag="psT", name=f"fill_{nfill[0]}", bufs=2)
        nfill[0] += 1
        nc.tensor.matmul(ft.rearrange("p a b -> p (a b)")[:, 0:256], lhsT=ident16,
                         rhs=kT[:, ec, 0:256], start=True, stop=True)
        nc.tensor.matmul(ft.rearrange("p a b -> p (a b)")[:, 256:512], lhsT=ident16,
                         rhs=kT[:, ec, 256:512], start=True, stop=True)

    # sim operands: nqT + first key half on sync, second half on scalar
    nqT = big.tile([P, DC, QS], f16, tag="nqT")
    nc.sync.dma_start(out=nqT, in_=nqtT_d.ap().rearrange("(dc p) j -> p dc j", p=P))
    nT = big.tile([P, DC, N], f16, tag="nT")
    n_r = ntT_d.ap().rearrange("(dc p) j -> p dc j", p=P)
    nc.sync.dma_start(out=nT[:, :, 0:1024], in_=n_r[:, :, 0:1024])
    nc.scalar.dma_start(out=nT[:, :, 1024:2048], in_=n_r[:, :, 1024:2048])
    xT = big.tile([P, DC, N], f16, tag="xT")
    nc.scalar.dma_start(out=xT, in_=xt16_d.ap().rearrange("(dc p) j -> p dc j", p=P))
    xqT = big.tile([P, DC, QS], f16, tag="xqT")
    nc.scalar.dma_start(out=xqT, in_=xqt16_d.ap().rearrange("(dc p) j -> p dc j", p=P))
    w16 = {}
    for nm in ("wk", "wq", "wv", "wo"):
        w16[nm] = consts.tile([P, DC, D], f16, tag=f"w16_{nm}", name=f"w16_{nm}")
        nc.gpsimd.dma_start(out=w16[nm],
                            in_=w_d[nm].ap().rearrange("(dc p) d -> p dc d", p=P))
    bkP = consts.tile([P, DC], f32, tag="bkP")
    nc.gpsimd.dma_start(out=bkP, in_=bkP_d[:, :])
    bqP = consts.tile([P, DC], f32, tag="bqP")
    nc.gpsimd.dma_start(out=bqP, in_=bqP_d[:, :])
    boP = consts.tile([P, DC], f32, tag="boP")
    nc.gpsimd.dma_start(out=boP, in_=boP_d[:, :])
    qv_sb = consts.tile([P, QT], f32, tag="qv_sb")
    nc.gpsimd.dma_start(out=qv_sb, in_=qv_d[:, :])
    g_rep = consts.tile([P, D], f32, tag="g_rep")
    nc.gpsimd.dma_start(out=g_rep, in_=bcast_ap(g_d, P))
    bt_rep = consts.tile([P, D], f32, tag="bt_rep")
    nc.gpsimd.dma_start(out=bt_rep, in_=bcast_ap(bt_d, P))
    xq_rows = big.tile([P, QT, D], f32, tag="xq_rows")
    nc.gpsimd.dma_start(out=xq_rows, in_=xq_d.ap().rearrange("(t p) d -> p t d", p=P))

    v_aug = big.tile([P, NT, H, DH + 1], f16, tag="v_aug")
    nc.gpsimd.memset(v_aug[:, :, :, DH:DH + 1], 1.0)

    dumm = small.tile([P, 8], f32, tag="dumm")
    nc.scalar.activation(dumm, ident[:, 0:8], AF.Exp)   # exp table preload

    # ---------------- sim (jg-major front half, K/Q ec0 between) -----
    kT = big.tile([P, DC, N], f16, tag="kT")
    qT = big.tile([P, DC, QS], f16, tag="qT")
    sims = [big.tile([P, N], f16, tag=f"sim_{t}", name=f"sim_{t}")
            for t in range(QT)]
    sim_ps = {}

    def sim_mm(t):
        # 4 single-bank psV tiles per query tile; copies run on DVE
        for jc in range(4):
            ps = psV.tile([P, 512], f32, tag="psV", name=f"psim_{t}_{jc}")
            sim_ps[(t, jc)] = ps
            for dc in range(DC):
                nc.tensor.matmul(
                    ps,
                    lhsT=nqT[:, dc, t * P:(t + 1) * P],
                    rhs=nT[:, dc, jc * 512:(jc + 1) * 512],
                    start=dc == 0, stop=dc == DC - 1)

    def sim_cp(t):
        for jc in range(4):
            nc.vector.tensor_copy(sims[t][:, jc * 512:(jc + 1) * 512],
                                  sim_ps[(t, jc)])

    def kproj_mm(ec, jg):
        pk = psA.tile([P, 2, 512], f32, tag="psA", name=f"pk_{ec}_{jg}")
        for k in range(2):
            jc = jg * 2 + k
            for dc in range(DC):
                nc.tensor.matmul(
                    pk[:, k, :],
                    lhsT=w16["wk"][:, dc, ec * P:(ec + 1) * P],
                    rhs=xT[:, dc, jc * 512:(jc + 1) * 512],
                    start=dc == 0, stop=dc == DC - 1)
        return pk

    def qproj_mm(ec):
        pq = psA.tile([P, 2, 512], f32, tag="psA", name=f"pq_{ec}")
        for dc in range(DC):
            nc.tensor.matmul(
                pq[:, 0, :],
                lhsT=w16["wq"][:, dc, ec * P:(ec + 1) * P],
                rhs=xqT[:, dc, :],
                start=dc == 0, stop=dc == DC - 1)
        return pq

    def kq_bias(ec):
        if not zb_qk:
            nc.gpsimd.tensor_scalar(kT[:, ec, :], kT[:, ec, :], bkP[:, ec:ec + 1],
                                    None, op0=OP.add)
            nc.gpsimd.tensor_scalar(qT[:, ec, :], qT[:, ec, :], bqP[:, ec:ec + 1],
                                    None, op0=OP.add)

    sim_mm(0)
    sim_mm(1)
    pks0 = [kproj_mm(0, jg) for jg in range(2)]
    pq0 = qproj_mm(0)
    for jg in range(2):
        nc.scalar.copy(kT[:, 0, jg * 1024:(jg + 1) * 1024], pks0[jg])
    nc.scalar.copy(qT[:, 0, :], pq0[:, 0, :])
    kq_bias(0)

    # ---------------- V projection ----------------
    pvs = {}

    def v_mm(jt):
        pv = psV.tile([P, 512], f32, tag="psV", name=f"pv_{jt}")
        pvs[jt] = pv
        for dc in range(DC):
            nc.tensor.matmul(
                pv[:, 0:D],
                lhsT=xT[:, dc, jt * P:(jt + 1) * P],
                rhs=w16["wv"][:, dc, :],
                start=dc == 0, stop=dc == DC - 1)

    def v_evac(jt):
        nc.vector.tensor_copy(v_aug[:, jt, :, 0:DH],
                              pvs[jt][:, 0:D].rearrange("p (h d) -> p h d", h=H))

    # ---------------- topk threshold + mask ----------------
    maskT = big.tile([P, NT, QS], f16, tag="maskT")

    def topk_tile(t):
        sim_m = sims[t]
        cand = scrp.tile([P, 64], f16, tag="cand", name=f"cand_{t}")
        for c in range(NCH):
            nc.vector.max(cand[:, c * 8:c * 8 + 8], sim_m[:, c * CH:(c + 1) * CH])
        mscr = scrp.tile([P, 64], f16, tag="mscr", name=f"mscr_{t}")
        mx = None
        for it in range(5):
            mx = small.tile([P, 8], f16, tag="mx8", name=f"mx_{t}_{it}")
            src = cand if it == 0 else mscr
            nc.vector.max(mx, src)
            if it < 4:
                nc.vector.match_replace(mscr, mx, src, MR_MIN)
        tS = small.tile([P, 1], f32, tag="tS", name=f"tS_{t}")
        tP_ = small.tile([P, 1], f32, tag="tP", name=f"tP_{t}")
        nc.vector.tensor_scalar(tP_, qv_sb[:, t:t + 1], 1.0, float(-TPAD),
                                op0=OP.subtract, op1=OP.mult)
        nc.vector.tensor_mul(tS, mx[:, 7:8], qv_sb[:, t:t + 1])
        nc.vector.tensor_add(tS, tS, tP_)
        mrow = scrp.tile([P, N], f16, tag="mrow", name=f"mrow_{t}")
        nc.vector.tensor_scalar(mrow, sim_m, tS, None, op0=OP.is_ge)
        nc.sync.dma_start_transpose(maskT[:, :, t * P:(t + 1) * P], mrow)

    # ---------------- scores/exp/mask/AV pipeline --------------------
    et = big.tile([P, NSLOT, 2, 512], f16, tag="et")
    outT = big.tile([P, DC, QS], f16, tag="outT")
    sumsAB = [big.tile([P, QS], f32, tag=f"sums_{e}", name=f"sums_{e}")
              for e in range(2)]
    pos = {}

    def slot(ec, jt, hp):
        return (ec * 2 * NT + 2 * jt + hp) % NSLOT

    def sc_exp(ec, jt, hp):
        ps = psA.tile([P, 2, 512], f32, tag="psA", name=f"sc_{ec}_{jt}_{hp}")
        for hl in range(2):
            r0 = 64 * hp + 32 * hl
            nc.tensor.matmul(
                ps[:, hl, :],
                lhsT=kT[r0:r0 + 32, ec, jt * P:(jt + 1) * P],
                rhs=qT[r0:r0 + 32, ec, :],
                start=True, stop=True, tile_position=(r0, 0))
        s = slot(ec, jt, hp)
        nc.scalar.activation(et[:, s, :, :], ps, AF.Exp, scale=float(SCALE))

    def mask_mult(ec, jt):
        m_b = rep_ap(maskT[:, jt, :], 1, 2)
        for hp in range(2):
            eng = nc.gpsimd if (hp == 0 and jt in GPS_JTS[ec]) else nc.vector
            s = slot(ec, jt, hp)
            eng.tensor_mul(et[:, s, :, :], et[:, s, :, :], m_b)

    def av(ec, jt):
        for hp in range(2):
            po = pos[(ec, hp)]
            s = slot(ec, jt, hp)
            for hl in range(2):
                h = 4 * ec + 2 * hp + hl
                nc.tensor.matmul(
                    po[64 * hl:64 * hl + DH + 1, :],
                    lhsT=v_aug[:, jt, h, :],
                    rhs=et[:, s, hl, :],
                    start=jt == 0, stop=jt == NT - 1,
                    tile_position=(0, 64 * hl), skip_group_check=True)

    def ec_evac_copies(ec, eng):
        # 8 copies: [hp0 outT+sums x2, then hp1], all shifts 32-aligned
        cps = []
        sums = sumsAB[ec]
        for hp in range(2):
            po = pos[(ec, hp)]
            for hl in range(2):
                hq = 2 * hp + hl
                cps.append((outT[32 * hq:32 * hq + 32, ec, :],
                            po[64 * hl:64 * hl + 32, :]))
                cps.append((sums[32 * hq:32 * hq + 1, :],
                            po[64 * hl + 32:64 * hl + 33, :]))
        return [(eng, dst, src) for dst, src in cps]

    sums_rows = big.tile([P, QT, H], f32, tag="sums_rows")
    recip_rows = big.tile([P, QT, H], f32, tag="recip_rows")
    out_rows = [big.tile([P, QT, P], f16, tag=f"our_{e}", name=f"our_{e}")
                for e in range(2)]
    outN = big.tile([P, DC, QS], f16, tag="outN")

    def ec_rows(ec):
        pt_s = psT.tile([P, 4, P], f32, tag="psT", name=f"pt_s_{ec}")
        for it in range(QT):
            nc.tensor.transpose(pt_s[:, it, :], sumsAB[ec][:, it * P:(it + 1) * P],
                                ident)
        for it in range(QT):
            base = pt_s[:, it, :]
            src = bass.AP(tensor=base.tensor, offset=base.offset,
                          ap=[list(base.ap[0]), [DH, 4]])
            nc.scalar.copy(sums_rows[:, it, 4 * ec:4 * ec + 4], src)
        pt_o = psT.tile([P, 4, P], f16, tag="psT", name=f"pt_o_{ec}")
        for it in range(QT):
            nc.tensor.transpose(pt_o[:, it, :],
                                outT[:, ec, it * P:(it + 1) * P], ident16)
        nc.vector.tensor_copy(out_rows[ec], pt_o)

    def ec_norm(ec):
        nc.vector.reciprocal(recip_rows[:, :, 4 * ec:4 * ec + 4],
                             sums_rows[:, :, 4 * ec:4 * ec + 4])
        rb = recip_rows[:, :, 4 * ec:4 * ec + 4]
        rb_b = bass.AP(tensor=rb.tensor, offset=rb.offset,
                       ap=[list(rb.ap[0]), list(rb.ap[1]), list(rb.ap[2]), [0, DH]])
        o = out_rows[ec]
        o_v = bass.AP(tensor=o.tensor, offset=o.offset,
                      ap=[list(o.ap[0]), list(o.ap[1]), [DH, 4], [1, DH]])
        nc.vector.tensor_mul(o_v, o_v, rb_b)

    def ec_outN(ec):
        pt = psT.tile([P, 4, P], f16, tag="psT", name=f"pt_n_{ec}")
        for it in range(QT):
            nc.tensor.transpose(pt[:, it, :], out_rows[ec][:, it, :], ident16)
        nc.scalar.copy(outN[:, ec, :], pt)

    # -- emission --
    # DVE FIFO: sim copies -> topk -> v_evacs -> ec0 masks -> (evacs0 + ec1
    # masks).  ACT FIFO: kT0/qT0 -> exp(ec0 stream) -> kT1/qT1 -> exp(ec1).
    # PE FIFO: all ec0 scores first (full lag: no AV blocks the exp stream),
    # ec1 scores interleaved into the ec0 mask/AV drain.
    sim_cp(0)
    sim_cp(1)
    for j0 in range(NT):
        sc_exp(0, j0, 0); sc_exp(0, j0, 1)
        if j0 == 2:
            sim_mm(2)
            sim_mm(3)
            sim_cp(2)
            sim_cp(3)
            for t in range(QT):
                topk_tile(t)
        if 3 <= j0 <= 8:
            for jv in range(3 * (j0 - 3), min(3 * (j0 - 3) + 3, NT)):
                v_mm(jv)
        if j0 == 4:
            pks1 = [kproj_mm(1, jg) for jg in range(2)]
            pq1 = qproj_mm(1)
            for jg in range(2):
                nc.scalar.copy(kT[:, 1, jg * 1024:(jg + 1) * 1024], pks1[jg])
            nc.scalar.copy(qT[:, 1, :], pq1[:, 0, :])
            kq_bias(1)
    for hp in range(2):
        pos[(0, hp)] = psV.tile([P, 512], f32, tag="psV", name=f"po_0_{hp}")
    for j0 in range(6):
        sc_exp(1, j0, 0); sc_exp(1, j0, 1)
    for jt in range(NT):
        if jt < 4:
            for jv in range(4 * jt, 4 * jt + 4):
                v_evac(jv)
        mask_mult(0, jt)
        av(0, jt)
        if jt + 6 < NT:
            sc_exp(1, jt + 6, 0); sc_exp(1, jt + 6, 1)
    for hp in range(2):
        pos[(1, hp)] = psV.tile([P, 512], f32, tag="psV", name=f"po_1_{hp}")
    evacs0 = ec_evac_copies(0, "dve")
    for jt in range(NT):
        mask_mult(1, jt)
        if jt < 4:
            for _, dst, srcc in evacs0[2 * jt:2 * jt + 2]:
                nc.vector.tensor_copy(dst, srcc)
        av(1, jt)
        if jt == 4:
            ec_rows(0)
        elif jt == 6:
            ec_norm(0)
        elif jt == 8:
            ec_outN(0)
    for eng, dst, srcc in ec_evac_copies(1, "act"):
        nc.scalar.copy(dst, srcc)
    ec_rows(1)
    ec_norm(1)
    ec_outN(1)

    # ---------------- out-proj, residual, LN --------------
    finalT = big.tile([P, DC, QS], f32, tag="finalT")
    pf = psA.tile([P, 2, 512], f32, tag="psA", name="pf")
    for eo in range(DC):
        for dc in range(DC):
            nc.tensor.matmul(
                pf[:, eo, :],
                lhsT=w16["wo"][:, dc, eo * P:(eo + 1) * P],
                rhs=outN[:, dc, :],
                start=dc == 0, stop=dc == DC - 1)
        nc.scalar.copy(finalT[:, eo, :], pf[:, eo, :])
        if not zb_o:
            nc.vector.tensor_scalar(finalT[:, eo, :], finalT[:, eo, :],
                                    boP[:, eo:eo + 1], None, op0=OP.add)

    fin = big.tile([P, QT, D], f32, tag="fin")
    for eo in range(DC):
        pt = psT.tile([P, 4, P], f32, tag="psT", name=f"ptf_{eo}")
        for it in range(QT):
            nc.tensor.transpose(pt[:, it, :], finalT[:, eo, it * P:(it + 1) * P],
                                ident)
        nc.scalar.copy(fin[:, 0:QT, eo * P:(eo + 1) * P], pt)
    nc.vector.tensor_add(fin, fin, xq_rows)

    st6 = small.tile([P, QT, 6], f32, tag="st6")
    mv = small.tile([P, QT, 2], f32, tag="mv")
    for t in range(QT):
        nc.vector.bn_stats(st6[:, t, :], fin[:, t, :])
        nc.vector.bn_aggr(mv[:, t, :], st6[:, t, :])
    rstd = small.tile([P, QT, 1], f32, tag="rstd")
    nc.vector.tensor_scalar(rstd, mv[:, :, 1:2], float(LN_EPS), None, op0=OP.add)
    nc.scalar.activation(rstd, rstd, AF.Sqrt)
    nc.vector.reciprocal(rstd, rstd)
    for t in range(QT):
        nc.vector.tensor_scalar(fin[:, t, :], fin[:, t, :], mv[:, t, 0:1],
                                rstd[:, t, 0:1], op0=OP.subtract, op1=OP.mult)
        if not ln_id:
            nc.vector.tensor_mul(fin[:, t, :], fin[:, t, :], g_rep)
            nc.vector.tensor_add(fin[:, t, :], fin[:, t, :], bt_rep)
        nc.sync.dma_start(out=out_d[t * P:(t + 1) * P, :], in_=fin[:, t, :])


def build_nc(zb_qk=True, zb_o=True, ln_id=True):
    from contextlib import ExitStack
    import concourse.bacc as bacc
    from concourse.tile import TileContext

    nc = bacc.Bacc("TRN2", target_bir_lowering=False, debug=False, num_devices=NCORES)
    with TileContext(nc) as tc:
        with ExitStack() as ctx:
            _emit(nc, tc, ctx, zb_qk, zb_o, ln_id)
    nc.compile()
    return nc


def _in_maps(inputs):
    x = np.ascontiguousarray(np.asarray(inputs["stock_features"], dtype=np.float32))
    valid = np.asarray(inputs["stock_valid_mask"]).astype(bool)
    wq = np.asarray(inputs["w_q"], np.float32)
    wk = np.asarray(inputs["w_k"], np.float32)
    wv = np.asarray(inputs["w_v"], np.float32)
    wo = np.asarray(inputs["w_o"], np.float32)
    bo_f = (np.asarray(inputs["b_v"], np.float32) @ wo
            + np.asarray(inputs["b_o"], np.float32))
    shared = {
        "wq16": np.ascontiguousarray(wq.astype(np.float16)),
        "wk16": np.ascontiguousarray(wk.astype(np.float16)),
        "wv16": np.ascontiguousarray(wv.astype(np.float16)),
        "wo16": np.ascontiguousarray(wo.astype(np.float16)),
        "bkP": np.ascontiguousarray(
            np.asarray(inputs["b_k"], np.float32).reshape(DC, P).T),
        "bqP": np.ascontiguousarray(
            np.asarray(inputs["b_q"], np.float32).reshape(DC, P).T),
        "boP": np.ascontiguousarray(bo_f.reshape(DC, P).T),
        "g": np.ascontiguousarray(inputs["ln_g"], np.float32),
        "bt": np.ascontiguousarray(inputs["ln_b"], np.float32),
    }
    maps = []
    for c in range(NCORES):
        b, qi = divmod(c, 4)
        q0 = qi * QS
        qv = valid[b, q0:q0 + QS].astype(np.float32).reshape(QT, P).T.copy()
        m = dict(shared)
        m["xq"] = np.ascontiguousarray(x[b, q0:q0 + QS])
        xt16 = np.ascontiguousarray(x[b].T.astype(np.float16))
        m["xt16"] = xt16
        m["xqt16"] = np.ascontiguousarray(xt16[:, q0:q0 + QS])
        n = x[b] / np.clip(np.linalg.norm(x[b], axis=-1, keepdims=True), 1e-12, None)
        n[~valid[b]] = 0.0
        ntT = np.ascontiguousarray(n.T.astype(np.float16))
        m["ntT"] = ntT
        m["nqtT"] = np.ascontiguousarray(ntT[:, q0:q0 + QS])
        m["qv"] = qv
        maps.append(m)
    return maps


def kernel(**inputs):
    from concourse.bass_utils import run_bass_kernel_spmd

    zb_qk = (not np.any(np.asarray(inputs["b_q"]))
             and not np.any(np.asarray(inputs["b_k"])))
    bo_f = (np.asarray(inputs["b_v"], np.float32)
            @ np.asarray(inputs["w_o"], np.float32)
            + np.asarray(inputs["b_o"], np.float32))
    zb_o = not np.any(bo_f)
    ln_id = (np.all(np.asarray(inputs["ln_g"]) == 1.0)
             and not np.any(np.asarray(inputs["ln_b"])))
    key = ("nc", zb_qk, zb_o, ln_id)
    if key not in _CACHE:
        _CACHE[key] = build_nc(zb_qk, zb_o, ln_id)
    nc = _CACHE[key]
    res = run_bass_kernel_spmd(nc, _in_maps(inputs), list(range(NCORES)))
    out = np.empty((B, N, D), np.float32)
    for c in range(NCORES):
        b, qi = divmod(c, 4)
        out[b, qi * QS:(qi + 1) * QS] = res.results[c]["out"]
    return out


# revision 16
# speedup vs baseline: 1.1097x; 1.0691x over previous
"""Trainium2 Bass kernel for CrossStockAttention (sparse similarity-top-k attention).

Sharding: 8 cores = 2 batches x 4 query-row blocks of 512. Each core receives
the full key set of its batch plus its query slice, computes sim -> chunked
top-40 threshold -> mask -> QKV -> masked softmax attention -> out-proj ->
residual -> LayerNorm for its 512 rows on-chip.

v4 (from the 166us v3):
 - PE filler matmuls (1 x N=512 per key-tile) bridge the exp-paced PE idle
   gaps so the HAM un-throttles; v3 ran the whole stream at K=4/8 (600ns
   matmuls) because PE bursts never reached a full 3.4us activity window
 - zero-bias / identity-LN specialization decided at kernel() time from the
   actual inputs (spec fills are zeros/ones) -> no bias ts ops, no LN g/b
   tensor_tensor passes; non-zero inputs rebuild with the general path
 - dummy sqrt removed (Tile hoisted it early, evicting the exp ACT table
   mid-front and forcing a reload before the first real exp)
 - sim emitted jg-major with K/Q ec0 projections between, weights DMA moved
   to the gpsimd queue, nT split sync/scalar -> first exp ~14us earlier
 - ec0 po evacuation interleaved into the early ec1 mask stream (hp0 first)
   instead of one 5us DVE block at the boundary
"""

import numpy as np

B, N, D, H = 2, 2048, 256, 8
DH = D // H            # 32
TOPK = 40
P = 128
NCORES = 8
QS = 512               # query rows per core
NT = N // P            # 16 key row-tiles
QT = QS // P           # 4 query row-tiles
DC = D // P            # 2 contraction chunks of 128
LN_EPS = 1e-5
SCALE = 1.0 / DH ** 0.5
MR_MIN = -60000.0      # fp16-safe
TPAD = -100.0

CH = 256               # topk chunk width (8 chunks, 64 candidates)
NCH = N // CH
NSLOT = 40             # exp slab slots (20 key-tiles in flight)
GPS_JTS = {0: (3, 9), 1: (1, 3, 5, 7, 9, 11, 13)}  # hp0 mask-mult on gpsimd
N_WARM = 32
AV_LAG = 10

_CACHE = {}


def _emit(nc, tc, ctx, zb_qk, zb_o, ln_id):
    import concourse.bass as bass
    import concourse.mybir as mybir
    from concourse.masks import make_identity

    f32 = mybir.dt.float32
    f16 = mybir.dt.float16
    AF = mybir.ActivationFunctionType
    OP = mybir.AluOpType

    xq_d = nc.dram_tensor("xq", [QS, D], f32, kind="ExternalInput")
    xt16_d = nc.dram_tensor("xt16", [D, N], f16, kind="ExternalInput")
    xqt16_d = nc.dram_tensor("xqt16", [D, QS], f16, kind="ExternalInput")
    ntT_d = nc.dram_tensor("ntT", [D, N], f16, kind="ExternalInput")
    nqtT_d = nc.dram_tensor("nqtT", [D, QS], f16, kind="ExternalInput")
    w_d = {}
    for nm in ("wq", "wk", "wv", "wo"):
        w_d[nm] = nc.dram_tensor(nm + "16", [D, D], f16, kind="ExternalInput")
    bkP_d = nc.dram_tensor("bkP", [P, DC], f32, kind="ExternalInput")
    bqP_d = nc.dram_tensor("bqP", [P, DC], f32, kind="ExternalInput")
    boP_d = nc.dram_tensor("boP", [P, DC], f32, kind="ExternalInput")
    qv_d = nc.dram_tensor("qv", [P, QT], f32, kind="ExternalInput")
    g_d = nc.dram_tensor("g", [D], f32, kind="ExternalInput")
    bt_d = nc.dram_tensor("bt", [D], f32, kind="ExternalInput")
    out_d = nc.dram_tensor("out", [QS, D], f32, kind="ExternalOutput")

    def bcast_ap(handle, n_part):
        ap = handle.ap()
        return bass.AP(tensor=ap.tensor, offset=ap.offset,
                       ap=[[0, n_part]] + [list(p) for p in ap.ap])

    def rep_ap(ap_in, ins_pos, count):
        a = [list(p) for p in ap_in.ap]
        a.insert(ins_pos, [0, count])
        return bass.AP(tensor=ap_in.tensor, offset=ap_in.offset, ap=a)

    consts = ctx.enter_context(tc.tile_pool(name="consts", bufs=1))
    big = ctx.enter_context(tc.tile_pool(name="big", bufs=1))
    scrp = ctx.enter_context(tc.tile_pool(name="scrp", bufs=2))
    small = ctx.enter_context(tc.tile_pool(name="small", bufs=6))
    psA = ctx.enter_context(tc.tile_pool(name="psA", bufs=2, space="PSUM"))
    psV = ctx.enter_context(tc.tile_pool(name="psV", bufs=2, space="PSUM"))
    psT = ctx.enter_context(tc.tile_pool(name="psT", bufs=2, space="PSUM"))

    # ---------------- constants / warmup / input DMAs ----------------
    ident = consts.tile([P, P], f32, tag="ident")
    make_identity(nc, ident)
    ident16 = consts.tile([P, P], f16, tag="ident16")
    nc.vector.tensor_copy(ident16, ident)

    for w in range(N_WARM):
        wt = psT.tile([P, 4, P], f32, tag="psT", name=f"warm_{w}", bufs=2)
        nc.tensor.matmul(wt[:, 0, :], lhsT=ident16, rhs=ident16,
                         start=True, stop=True)

    nfill = [0]

    def filler(ec):
        # real matmuls (no deps beyond resident kT) to keep the HAM warm
        ft = psT.tile([P, 4, P], f32, tag="psT", name=f"fill_{nfill[0]}", bufs=2)
        nfill[0] += 1
        nc.tensor.matmul(ft.rearrange("p a b -> p (a b)")[:, 0:256], lhsT=ident16,
                         rhs=kT[:, ec, 0:256], start=True, stop=True)
        nc.tensor.matmul(ft.rearrange("p a b -> p (a b)")[:, 256:512], lhsT=ident16,
                         rhs=kT[:, ec, 256:512], start=True, stop=True)

    # sim operands: nqT + first key half on sync, second half on scalar
    nqT = big.tile([P, DC, QS], f16, tag="nqT")
    nc.sync.dma_start(out=nqT, in_=nqtT_d.ap().rearrange("(dc p) j -> p dc j", p=P))
    nT = big.tile([P, DC, N], f16, tag="nT")
    n_r = ntT_d.ap().rearrange("(dc p) j -> p dc j", p=P)
    nc.sync.dma_start(out=nT[:, :, 0:1024], in_=n_r[:, :, 0:1024])
    nc.scalar.dma_start(out=nT[:, :, 1024:2048], in_=n_r[:, :, 1024:2048])
    xT = big.tile([P, DC, N], f16, tag="xT")
    nc.scalar.dma_start(out=xT, in_=xt16_d.ap().rearrange("(dc p) j -> p dc j", p=P))
    xqT = big.tile([P, DC, QS], f16, tag="xqT")
    nc.scalar.dma_start(out=xqT, in_=xqt16_d.ap().rearrange("(dc p) j -> p dc j", p=P))
    w16 = {}
    for nm in ("wk", "wq", "wv", "wo"):
        w16[nm] = consts.tile([P, DC, D], f16, tag=f"w16_{nm}", name=f"w16_{nm}")
        nc.gpsimd.dma_start(out=w16[nm],
                            in_=w_d[nm].ap().rearrange("(dc p) d -> p dc d", p=P))
    bkP = consts.tile([P, DC], f32, tag="bkP")
    nc.gpsimd.dma_start(out=bkP, in_=bkP_d[:, :])
    bqP = consts.tile([P, DC], f32, tag="bqP")
    nc.gpsimd.dma_start(out=bqP, in_=bqP_d[:, :])
    boP = consts.tile([P, DC], f32, tag="boP")
    nc.gpsimd.dma_start(out=boP, in_=boP_d[:, :])
    qv_sb = consts.tile([P, QT], f32, tag="qv_sb")
    nc.gpsimd.dma_start(out=qv_sb, in_=qv_d[:, :])
    g_rep = consts.tile([P, D], f32, tag="g_rep")
    nc.gpsimd.dma_start(out=g_rep, in_=bcast_ap(g_d, P))
    bt_rep = consts.tile([P, D], f32, tag="bt_rep")
    nc.gpsimd.dma_start(out=bt_rep, in_=bcast_ap(bt_d, P))
    xq_rows = big.tile([P, QT, D], f32, tag="xq_rows")
    nc.gpsimd.dma_start(out=xq_rows, in_=xq_d.ap().rearrange("(t p) d -> p t d", p=P))

    v_aug = big.tile([P, NT, H, DH + 1], f16, tag="v_aug")
    nc.gpsimd.memset(v_aug[:, :, :, DH:DH + 1], 1.0)

    dumm = small.tile([P, 8], f32, tag="dumm")
    nc.scalar.activation(dumm, ident[:, 0:8], AF.Exp)   # exp table preload

    # ---------------- sim (jg-major front half, K/Q ec0 between) -----
    kT = big.tile([P, DC, N], f16, tag="kT")
    qT = big.tile([P, DC, QS], f16, tag="qT")
    sims = [big.tile([P, N], f16, tag=f"sim_{t}", name=f"sim_{t}")
            for t in range(QT)]
    sim_ps = {}

    def sim_mm(t):
        # 4 single-bank psV tiles per query tile; copies run on DVE
        for jc in range(4):
            ps = psV.tile([P, 512], f32, tag="psV", name=f"psim_{t}_{jc}")
            sim_ps[(t, jc)] = ps
            for dc in range(DC):
                nc.tensor.matmul(
                    ps,
                    lhsT=nqT[:, dc, t * P:(t + 1) * P],
                    rhs=nT[:, dc, jc * 512:(jc + 1) * 512],
                    start=dc == 0, stop=dc == DC - 1)

    def sim_cp(t):
        for jc in range(4):
            nc.vector.tensor_copy(sims[t][:, jc * 512:(jc + 1) * 512],
                                  sim_ps[(t, jc)])

    def kproj_mm(ec, jg):
        pk = psA.tile([P, 2, 512], f32, tag="psA", name=f"pk_{ec}_{jg}")
        for k in range(2):
            jc = jg * 2 + k
            for dc in range(DC):
                nc.tensor.matmul(
                    pk[:, k, :],
                    lhsT=w16["wk"][:, dc, ec * P:(ec + 1) * P],
                    rhs=xT[:, dc, jc * 512:(jc + 1) * 512],
                    start=dc == 0, stop=dc == DC - 1)
        return pk

    def qproj_mm(ec):
        pq = psA.tile([P, 2, 512], f32, tag="psA", name=f"pq_{ec}")
        for dc in range(DC):
            nc.tensor.matmul(
                pq[:, 0, :],
                lhsT=w16["wq"][:, dc, ec * P:(ec + 1) * P],
                rhs=xqT[:, dc, :],
                start=dc == 0, stop=dc == DC - 1)
        return pq

    def kq_bias(ec):
        if not zb_qk:
            nc.gpsimd.tensor_scalar(kT[:, ec, :], kT[:, ec, :], bkP[:, ec:ec + 1],
                                    None, op0=OP.add)
            nc.gpsimd.tensor_scalar(qT[:, ec, :], qT[:, ec, :], bqP[:, ec:ec + 1],
                                    None, op0=OP.add)

    sim_mm(0)
    sim_mm(1)
    pks0 = [kproj_mm(0, jg) for jg in range(2)]
    pq0 = qproj_mm(0)
    for jg in range(2):
        nc.scalar.copy(kT[:, 0, jg * 1024:(jg + 1) * 1024], pks0[jg])
    nc.scalar.copy(qT[:, 0, :], pq0[:, 0, :])
    kq_bias(0)

    # ---------------- V projection ----------------
    pvs = {}

    def v_mm(jt):
        pv = psV.tile([P, 512], f32, tag="psV", name=f"pv_{jt}")
        pvs[jt] = pv
        for dc in range(DC):
            nc.tensor.matmul(
                pv[:, 0:D],
                lhsT=xT[:, dc, jt * P:(jt + 1) * P],
                rhs=w16["wv"][:, dc, :],
                start=dc == 0, stop=dc == DC - 1)

    def v_evac(jt):
        nc.vector.tensor_copy(v_aug[:, jt, :, 0:DH],
                              pvs[jt][:, 0:D].rearrange("p (h d) -> p h d", h=H))

    # ---------------- topk threshold + mask ----------------
    maskT = big.tile([P, NT, QS], f16, tag="maskT")

    def topk_tile(t):
        sim_m = sims[t]
        cand = scrp.tile([P, 64], f16, tag="cand", name=f"cand_{t}")
        for c in range(NCH):
            nc.vector.max(cand[:, c * 8:c * 8 + 8], sim_m[:, c * CH:(c + 1) * CH])
        mscr = scrp.tile([P, 64], f16, tag="mscr", name=f"mscr_{t}")
        mx = None
        for it in range(5):
            mx = small.tile([P, 8], f16, tag="mx8", name=f"mx_{t}_{it}")
            src = cand if it == 0 else mscr
            nc.vector.max(mx, src)
            if it < 4:
                nc.vector.match_replace(mscr, mx, src, MR_MIN)
        tS = small.tile([P, 1], f32, tag="tS", name=f"tS_{t}")
        tP_ = small.tile([P, 1], f32, tag="tP", name=f"tP_{t}")
        nc.vector.tensor_scalar(tP_, qv_sb[:, t:t + 1], 1.0, float(-TPAD),
                                op0=OP.subtract, op1=OP.mult)
        nc.vector.tensor_mul(tS, mx[:, 7:8], qv_sb[:, t:t + 1])
        nc.vector.tensor_add(tS, tS, tP_)
        mrow = scrp.tile([P, N], f16, tag="mrow", name=f"mrow_{t}")
        nc.vector.tensor_scalar(mrow, sim_m, tS, None, op0=OP.is_ge)
        nc.sync.dma_start_transpose(maskT[:, :, t * P:(t + 1) * P], mrow)

    # ---------------- scores/exp/mask/AV pipeline --------------------
    et = big.tile([P, NSLOT, 2, 512], f16, tag="et")
    outT = big.tile([P, DC, QS], f16, tag="outT")
    sumsAB = [big.tile([P, QS], f32, tag=f"sums_{e}", name=f"sums_{e}")
              for e in range(2)]
    pos = {}

    def slot(ec, jt, hp):
        return (ec * 2 * NT + 2 * jt + hp) % NSLOT

    def sc_exp(ec, jt, hp):
        ps = psA.tile([P, 2, 512], f32, tag="psA", name=f"sc_{ec}_{jt}_{hp}")
        for hl in range(2):
            r0 = 64 * hp + 32 * hl
            nc.tensor.matmul(
                ps[:, hl, :],
                lhsT=kT[r0:r0 + 32, ec, jt * P:(jt + 1) * P],
                rhs=qT[r0:r0 + 32, ec, :],
                start=True, stop=True, tile_position=(r0, 0))
        s = slot(ec, jt, hp)
        nc.scalar.activation(et[:, s, :, :], ps, AF.Exp, scale=float(SCALE))

    def mask_mult(ec, jt):
        m_b = rep_ap(maskT[:, jt, :], 1, 2)
        for hp in range(2):
            eng = nc.gpsimd if (hp == 0 and jt in GPS_JTS[ec]) else nc.vector
            s = slot(ec, jt, hp)
            eng.tensor_mul(et[:, s, :, :], et[:, s, :, :], m_b)

    def av(ec, jt):
        for hp in range(2):
            po = pos[(ec, hp)]
            s = slot(ec, jt, hp)
            for hl in range(2):
                h = 4 * ec + 2 * hp + hl
                nc.tensor.matmul(
                    po[64 * hl:64 * hl + DH + 1, :],
                    lhsT=v_aug[:, jt, h, :],
                    rhs=et[:, s, hl, :],
                    start=jt == 0, stop=jt == NT - 1,
                    tile_position=(0, 64 * hl), skip_group_check=True)

    def ec_evac_copies(ec, eng):
        # 8 copies: [hp0 outT+sums x2, then hp1], all shifts 32-aligned
        cps = []
        sums = sumsAB[ec]
        for hp in range(2):
            po = pos[(ec, hp)]
            for hl in range(2):
                hq = 2 * hp + hl
                cps.append((outT[32 * hq:32 * hq + 32, ec, :],
                            po[64 * hl:64 * hl + 32, :]))
                cps.append((sums[32 * hq:32 * hq + 1, :],
                            po[64 * hl + 32:64 * hl + 33, :]))
        return [(eng, dst, src) for dst, src in cps]

    sums_rows = big.tile([P, QT, H], f32, tag="sums_rows")
    recip_rows = big.tile([P, QT, H], f32, tag="recip_rows")
    out_rows = [big.tile([P, QT, P], f16, tag=f"our_{e}", name=f"our_{e}")
                for e in range(2)]
    outN = big.tile([P, DC, QS], f16, tag="outN")

    def ec_rows(ec):
        pt_s = psT.tile([P, 4, P], f32, tag="psT", name=f"pt_s_{ec}")
        for it in range(QT):
            nc.tensor.transpose(pt_s[:, it, :], sumsAB[ec][:, it * P:(it + 1) * P],
                                ident)
        for it in range(QT):
            base = pt_s[:, it, :]
            src = bass.AP(tensor=base.tensor, offset=base.offset,
                          ap=[list(base.ap[0]), [DH, 4]])
            nc.scalar.copy(sums_rows[:, it, 4 * ec:4 * ec + 4], src)
        pt_o = psT.tile([P, 4, P], f16, tag="psT", name=f"pt_o_{ec}")
        for it in range(QT):
            nc.tensor.transpose(pt_o[:, it, :],
                                outT[:, ec, it * P:(it + 1) * P], ident16)
        nc.vector.tensor_copy(out_rows[ec], pt_o)

    def ec_norm(ec):
        nc.vector.reciprocal(recip_rows[:, :, 4 * ec:4 * ec + 4],
                             sums_rows[:, :, 4 * ec:4 * ec + 4])
        rb = recip_rows[:, :, 4 * ec:4 * ec + 4]
        rb_b = bass.AP(tensor=rb.tensor, offset=rb.offset,
                       ap=[list(rb.ap[0]), list(rb.ap[1]), list(rb.ap[2]), [0, DH]])
        o = out_rows[ec]
        o_v = bass.AP(tensor=o.tensor, offset=o.offset,
                      ap=[list(o.ap[0]), list(o.ap[1]), [DH, 4], [1, DH]])
        nc.vector.tensor_mul(o_v, o_v, rb_b)

    def ec_outN(ec):
        pt = psT.tile([P, 4, P], f16, tag="psT", name=f"pt_n_{ec}")
        for it in range(QT):
            nc.tensor.transpose(pt[:, it, :], out_rows[ec][:, it, :], ident16)
        nc.scalar.copy(outN[:, ec, :], pt)

    # -- emission --
    # DVE FIFO: sim copies -> topk -> v_evacs -> ec0 masks -> (evacs0 + ec1
    # masks).  ACT FIFO: kT0/qT0 -> exp(ec0 stream) -> kT1/qT1 -> exp(ec1).
    # PE FIFO: all ec0 scores first (full lag: no AV blocks the exp stream),
    # ec1 scores interleaved into the ec0 mask/AV drain.
    sim_cp(0)
    sim_cp(1)
    for j0 in range(NT):
        sc_exp(0, j0, 0); sc_exp(0, j0, 1)
        if j0 == 2:
            sim_mm(2)
            sim_mm(3)
            sim_cp(2)
            sim_cp(3)
            for t in range(QT):
                topk_tile(t)
        if 3 <= j0 <= 8:
            for jv in range(3 * (j0 - 3), min(3 * (j0 - 3) + 3, NT)):
                v_mm(jv)
        if j0 == 4:
            pks1 = [kproj_mm(1, jg) for jg in range(2)]
            pq1 = qproj_mm(1)
            for jg in range(2):
                nc.scalar.copy(kT[:, 1, jg * 1024:(jg + 1) * 1024], pks1[jg])
            nc.scalar.copy(qT[:, 1, :], pq1[:, 0, :])
            kq_bias(1)
    for hp in range(2):
        pos[(0, hp)] = psV.tile([P, 512], f32, tag="psV", name=f"po_0_{hp}")
    for j0 in range(4):
        sc_exp(1, j0, 0); sc_exp(1, j0, 1)
    for jt in range(NT):
        if jt < 4:
            for jv in range(4 * jt, 4 * jt + 4):
                v_evac(jv)
        mask_mult(0, jt)
        av(0, jt)
        # slot(1, jt+4) aliases slot(0, jt): emit only after av(0, jt) so the
        # WAR on the et slot is tracked in program order
        if jt + 4 < NT:
            sc_exp(1, jt + 4, 0); sc_exp(1, jt + 4, 1)
    for hp in range(2):
        pos[(1, hp)] = psV.tile([P, 512], f32, tag="psV", name=f"po_1_{hp}")
    evacs0 = ec_evac_copies(0, "dve")
    for jt in range(NT):
        mask_mult(1, jt)
        if jt < 4:
            for _, dst, srcc in evacs0[2 * jt:2 * jt + 2]:
                nc.vector.tensor_copy(dst, srcc)
        av(1, jt)
        if jt == 4:
            ec_rows(0)
        elif jt == 6:
            ec_norm(0)
        elif jt == 8:
            ec_outN(0)
    for eng, dst, srcc in ec_evac_copies(1, "act"):
        nc.scalar.copy(dst, srcc)
    ec_rows(1)
    ec_norm(1)
    ec_outN(1)

    # ---------------- out-proj, residual, LN --------------
    finalT = big.tile([P, DC, QS], f32, tag="finalT")
    pf = psA.tile([P, 2, 512], f32, tag="psA", name="pf")
    for eo in range(DC):
        for dc in range(DC):
            nc.tensor.matmul(
                pf[:, eo, :],
                lhsT=w16["wo"][:, dc, eo * P:(eo + 1) * P],
                rhs=outN[:, dc, :],
                start=dc == 0, stop=dc == DC - 1)
        nc.scalar.copy(finalT[:, eo, :], pf[:, eo, :])
        if not zb_o:
            nc.vector.tensor_scalar(finalT[:, eo, :], finalT[:, eo, :],
                                    boP[:, eo:eo + 1], None, op0=OP.add)

    fin = big.tile([P, QT, D], f32, tag="fin")
    for eo in range(DC):
        pt = psT.tile([P, 4, P], f32, tag="psT", name=f"ptf_{eo}")
        for it in range(QT):
            nc.tensor.transpose(pt[:, it, :], finalT[:, eo, it * P:(it + 1) * P],
                                ident)
        nc.scalar.copy(fin[:, 0:QT, eo * P:(eo + 1) * P], pt)
    nc.vector.tensor_add(fin, fin, xq_rows)

    st6 = small.tile([P, QT, 6], f32, tag="st6")
    mv = small.tile([P, QT, 2], f32, tag="mv")
    for t in range(QT):
        nc.vector.bn_stats(st6[:, t, :], fin[:, t, :])
        nc.vector.bn_aggr(mv[:, t, :], st6[:, t, :])
    rstd = small.tile([P, QT, 1], f32, tag="rstd")
    nc.vector.tensor_scalar(rstd, mv[:, :, 1:2], float(LN_EPS), None, op0=OP.add)
    nc.scalar.activation(rstd, rstd, AF.Sqrt)
    nc.vector.reciprocal(rstd, rstd)
    for t in range(QT):
        nc.vector.tensor_scalar(fin[:, t, :], fin[:, t, :], mv[:, t, 0:1],
                                rstd[:, t, 0:1], op0=OP.subtract, op1=OP.mult)
        if not ln_id:
            nc.vector.tensor_mul(fin[:, t, :], fin[:, t, :], g_rep)
            nc.vector.tensor_add(fin[:, t, :], fin[:, t, :], bt_rep)
        nc.sync.dma_start(out=out_d[t * P:(t + 1) * P, :], in_=fin[:, t, :])


def build_nc(zb_qk=True, zb_o=True, ln_id=True):
    from contextlib import ExitStack
    import concourse.bacc as bacc
    from concourse.tile import TileContext

    nc = bacc.Bacc("TRN2", target_bir_lowering=False, debug=False, num_devices=NCORES)
    with TileContext(nc) as tc:
        with ExitStack() as ctx:
            _emit(nc, tc, ctx, zb_qk, zb_o, ln_id)
    nc.compile()
    return nc


def _in_maps(inputs):
    x = np.ascontiguousarray(np.asarray(inputs["stock_features"], dtype=np.float32))
    valid = np.asarray(inputs["stock_valid_mask"]).astype(bool)
    wq = np.asarray(inputs["w_q"], np.float32)
    wk = np.asarray(inputs["w_k"], np.float32)
    wv = np.asarray(inputs["w_v"], np.float32)
    wo = np.asarray(inputs["w_o"], np.float32)
    bo_f = (np.asarray(inputs["b_v"], np.float32) @ wo
            + np.asarray(inputs["b_o"], np.float32))
    shared = {
        "wq16": np.ascontiguousarray(wq.astype(np.float16)),
        "wk16": np.ascontiguousarray(wk.astype(np.float16)),
        "wv16": np.ascontiguousarray(wv.astype(np.float16)),
        "wo16": np.ascontiguousarray(wo.astype(np.float16)),
        "bkP": np.ascontiguousarray(
            np.asarray(inputs["b_k"], np.float32).reshape(DC, P).T),
        "bqP": np.ascontiguousarray(
            np.asarray(inputs["b_q"], np.float32).reshape(DC, P).T),
        "boP": np.ascontiguousarray(bo_f.reshape(DC, P).T),
        "g": np.ascontiguousarray(inputs["ln_g"], np.float32),
        "bt": np.ascontiguousarray(inputs["ln_b"], np.float32),
    }
    maps = []
    for c in range(NCORES):
        b, qi = divmod(c, 4)
        q0 = qi * QS
        qv = valid[b, q0:q0 + QS].astype(np.float32).reshape(QT, P).T.copy()
        m = dict(shared)
        m["xq"] = np.ascontiguousarray(x[b, q0:q0 + QS])
        xt16 = np.ascontiguousarray(x[b].T.astype(np.float16))
        m["xt16"] = xt16
        m["xqt16"] = np.ascontiguousarray(xt16[:, q0:q0 + QS])
        n = x[b] / np.clip(np.linalg.norm(x[b], axis=-1, keepdims=True), 1e-12, None)
        n[~valid[b]] = 0.0
        ntT = np.ascontiguousarray(n.T.astype(np.float16))
        m["ntT"] = ntT
        m["nqtT"] = np.ascontiguousarray(ntT[:, q0:q0 + QS])
        m["qv"] = qv
        maps.append(m)
    return maps


def kernel(**inputs):
    from concourse.bass_utils import run_bass_kernel_spmd

    zb_qk = (not np.any(np.asarray(inputs["b_q"]))
             and not np.any(np.asarray(inputs["b_k"])))
    bo_f = (np.asarray(inputs["b_v"], np.float32)
            @ np.asarray(inputs["w_o"], np.float32)
            + np.asarray(inputs["b_o"], np.float32))
    zb_o = not np.any(bo_f)
    ln_id = (np.all(np.asarray(inputs["ln_g"]) == 1.0)
             and not np.any(np.asarray(inputs["ln_b"])))
    key = ("nc", zb_qk, zb_o, ln_id)
    if key not in _CACHE:
        _CACHE[key] = build_nc(zb_qk, zb_o, ln_id)
    nc = _CACHE[key]
    res = run_bass_kernel_spmd(nc, _in_maps(inputs), list(range(NCORES)))
    out = np.empty((B, N, D), np.float32)
    for c in range(NCORES):
        b, qi = divmod(c, 4)
        out[b, qi * QS:(qi + 1) * QS] = res.results[c]["out"]
    return out
